# revision 1
# baseline (speedup 1.0000x reference)
"""TRN2 Bass kernel for DenseDilatedKnnGraph (B=4, C=64, N=4096, k=9, dilation=2).

Algorithm
---------
reference: xt (B,N,C); dist(i,j) = |xi|^2 - 2<xi,xj> + |xj|^2; nn_idx = top-18
of -dist per row (stable, lowest-index tie-break); output nn_idx[..., ::2] plus
a center-index row -> (2, B, N, 9) int32.

Per-row ordering of -dist is identical to the ordering of
    s_ij = 2<xi,xj> - |xj|^2
(the |xi|^2 term is constant per row), and s has better relative precision.

Device (per core, SPMD over 8 cores; core = (batch, query-half)):
  - s computed via 2 fp16 K=128 matmuls (hi/lo split of fp32, error ~1e-6,
    ~4x cheaper than native fp32 matmul on the PE; K=128 keeps the PE at
    1 cycle/column — K<=64 matmuls stream at half rate):
      s = (qh@ch + ql@ch) + (qh@cl + s1+s2+s3)
    matmul A: stationary [qh; ql] (128 x 128), moving [ch; ch] (128 x 512)
    matmul B: stationary [qh; 1,1,1, 0...] , moving [cl; s1; s2; s3; junk]
    where qh/ql = fp16 split of 2x (queries), ch/cl = fp16 split of x
    (candidates), s1..s3 = 3-level fp16 split of -|xj|^2. The zero rows of
    B's stationary null out the junk rows of its moving operand. PSUM fp32
    accumulate, 128-query tiles, 512-wide PSUM chunks.
  - PSUM -> SBUF copy on the scalar engine.
  - DVE top-k: per GROUP-wide group max8 (values) + max_index (local indices,
    first-occurrence = lowest-index tie-break, matching jax.lax.top_k).
  - DMA out: group-candidate values U (128 x UW), local indices L (128 x UW).

Host: one stable argsort of each row's UW group-candidates (slot order ==
global index order for equal values, preserving the stable tie-break) yields
the top-18 global indices per row; this merge is 64->18 bookkeeping on
device-selected candidates (the 4096->UW selection ran on device). Rows where
a single group contributed 8 members to the top-18 (its 9th member could have
been lost) are recomputed exactly on the host (~1300 of 16384 rows).
"""

import numpy as np

import concourse.bacc as bacc
import concourse.mybir as mybir
import concourse.tile as tile
from concourse.bass_utils import run_bass_kernel_spmd

# Problem constants (hardcoded per harness contract).
B = 4
C = 64
N = 4096
K = 9
DILATION = 2
K_EFF = K * DILATION      # 18
P = 128                   # partitions / queries per tile
KM = 128                  # matmul contraction (keeps PE in full-rate mode)
# DVE max8 group boundaries. Fewer/wider groups cut per-instruction DVE
# overhead (the 2 full passes over S are fixed cost) but raise the rate of
# hazard rows (a group contributing 8 of the top-18 needs a host recompute):
# 8x512 -> 114 rows (185.6us), 6x~683 -> 526 (179.3us), 5x~820 -> 1281
# (175.3us), 4x1024 -> 3726 rows of 16384 (171.4us, rejected: 23% repairs).
GROUP_BOUNDS = (0, 820, 1640, 2460, 3280, 4096)
NG = len(GROUP_BOUNDS) - 1
UW = NG * 8               # group-candidates per row
N_CORES = 8
QROWS = (B * N) // N_CORES          # 2048 query rows per core
N_TILES = QROWS // P                # 16 tiles per core


def _build_program(n_tiles=N_TILES):
    nc = bacc.Bacc(
        "TRN2", target_bir_lowering=False, debug=False, enable_asserts=False
    )
    f32 = mybir.dt.float32
    f16 = mybir.dt.float16
    u32 = mybir.dt.uint32
    nq = n_tiles * P
    lhs_a = nc.dram_tensor("lhs_a", (KM, nq), f16, kind="ExternalInput")
    lhs_b = nc.dram_tensor("lhs_b", (KM, nq), f16, kind="ExternalInput")
    rhs_a = nc.dram_tensor("rhs_a", (KM, N), f16, kind="ExternalInput")
    rhs_b = nc.dram_tensor("rhs_b", (KM, N), f16, kind="ExternalInput")
    u_out = nc.dram_tensor("u_out", (nq, UW), f32, kind="ExternalOutput")
    l_out = nc.dram_tensor("l_out", (nq, UW), u32, kind="ExternalOutput")
    lhs_a_ap, lhs_b_ap = lhs_a.ap(), lhs_b.ap()
    rhs_a_ap, rhs_b_ap = rhs_a.ap(), rhs_b.ap()
    u_ap, l_ap = u_out.ap(), l_out.ap()

    with tile.TileContext(nc) as tc:
        with (
            tc.tile_pool(name="const", bufs=1) as cpool,
            tc.tile_pool(name="psum", bufs=2, space="PSUM") as ppool,
            tc.tile_pool(name="work", bufs=4) as wpool,
            tc.tile_pool(name="outp", bufs=4) as opool,
        ):
            # dependency-free warm-up matmuls that run during the input-DMA
            # prologue (nudges the PE toward its full-rate mode before the
            # real K=128 stream starts; otherwise free)
            prime = cpool.tile([KM, 512], f16)
            nc.gpsimd.memset(prime[:, :], 0.0)
            pps = ppool.tile([P, N // 2], f32, tag="ps")
            for _ in range(12):
                nc.tensor.matmul(pps[:, :512], prime[:, :128], prime[:, :],
                                 start=True, stop=True)

            # per-512-column-chunk input tiles: the first matmul only waits
            # for its own 128KB chunk, not the whole 2MB load
            ra_sb = [
                cpool.tile([KM, 512], f16, name=f"ra{j}", tag=f"ra{j}")
                for j in range(8)
            ]
            rb_sb = [
                cpool.tile([KM, 512], f16, name=f"rb{j}", tag=f"rb{j}")
                for j in range(8)
            ]
            la_sb = cpool.tile([KM, nq], f16)
            lb_sb = cpool.tile([KM, nq], f16)
            # tile 0 needs la/lb chunk 0 + ra0/rb0 first; issue those before
            # the rest so the first matmul isn't gated on the whole load
            w0 = min(512, nq)
            nc.sync.dma_start(la_sb[:, 0:w0], lhs_a_ap[:, 0:w0])
            nc.sync.dma_start(lb_sb[:, 0:w0], lhs_b_ap[:, 0:w0])
            nc.sync.dma_start(ra_sb[0][:, :], rhs_a_ap[:, 0:512])
            nc.sync.dma_start(rb_sb[0][:, :], rhs_b_ap[:, 0:512])
            for j in range(1, 8):
                nc.sync.dma_start(ra_sb[j][:, :], rhs_a_ap[:, j * 512 : (j + 1) * 512])
                nc.sync.dma_start(rb_sb[j][:, :], rhs_b_ap[:, j * 512 : (j + 1) * 512])
            for j in range(512, nq, 512):
                w = min(512, nq - j)
                nc.sync.dma_start(la_sb[:, j : j + w], lhs_a_ap[:, j : j + w])
                nc.sync.dma_start(lb_sb[:, j : j + w], lhs_b_ap[:, j : j + w])

            for t in range(n_tiles):
                ssb = wpool.tile([P, N], f32, tag="ssb")
                qs = slice(t * P, (t + 1) * P)
                for h in range(2):
                    ps = ppool.tile([P, N // 2], f32, tag="ps")
                    for j in range(4):
                        cj = h * 4 + j
                        pslice = ps[:, j * 512 : (j + 1) * 512]
                        nc.tensor.matmul(
                            pslice, la_sb[:, qs], ra_sb[cj][:, :],
                            start=True, stop=False,
                        )
                        nc.tensor.matmul(
                            pslice, lb_sb[:, qs], rb_sb[cj][:, :],
                            start=False, stop=True,
                        )
                    for cc in range(4):
                        nc.scalar.copy(
                            ssb[:, h * (N // 2) + cc * 512 : h * (N // 2) + (cc + 1) * 512],
                            ps[:, cc * 512 : (cc + 1) * 512],
                        )

                u = opool.tile([P, UW], f32, tag="u")
                l = opool.tile([P, UW], u32, tag="l")
                for g in range(NG):
                    nc.vector.max(
                        out=u[:, g * 8 : (g + 1) * 8],
                        in_=ssb[:, GROUP_BOUNDS[g] : GROUP_BOUNDS[g + 1]],
                    )
                for g in range(NG):
                    nc.vector.max_index(
                        out=l[:, g * 8 : (g + 1) * 8],
                        in_max=u[:, g * 8 : (g + 1) * 8],
                        in_values=ssb[:, GROUP_BOUNDS[g] : GROUP_BOUNDS[g + 1]],
                    )

                rs = slice(t * P, (t + 1) * P)
                nc.sync.dma_start(u_ap[rs, :], u[:])
                nc.sync.dma_start(l_ap[rs, :], l[:])
    nc.compile()
    return nc


def _split16(a):
    hi = a.astype(np.float16)
    lo = (a - hi.astype(np.float32)).astype(np.float16)
    return hi, lo


def _prep_core_inputs(X, core):
    """X: (B, N, C) fp32. Returns input map for one core."""
    b, h = divmod(core, N_CORES // B)
    Xb = X[b]
    xsq = np.sum(Xb * Xb, axis=1, dtype=np.float32)
    ch, cl = _split16(Xb.T)                       # (C, N) fp16 each
    # 3-level fp16 split of -xsq
    s1 = (-xsq).astype(np.float16)
    r = -xsq - s1.astype(np.float32)
    s2 = r.astype(np.float16)
    s3 = (r - s2.astype(np.float32)).astype(np.float16)
    # matmul A: s_partial = qh@ch + ql@ch ; moving = [ch; ch]
    rhs_a = np.empty((KM, N), np.float16)
    rhs_a[:C] = ch
    rhs_a[C:] = ch
    # matmul B: += qh@cl + s1+s2+s3 ; moving = [cl; s1; s2; s3; zeros]
    rhs_b = np.zeros((KM, N), np.float16)
    rhs_b[:C] = cl
    rhs_b[C] = s1
    rhs_b[C + 1] = s2
    rhs_b[C + 2] = s3

    Q = 2.0 * Xb[h * QROWS : (h + 1) * QROWS]     # (QROWS, C)
    qh, ql = _split16(Q.T)                        # (C, QROWS)
    lhs_a = np.empty((KM, QROWS), np.float16)
    lhs_a[:C] = qh
    lhs_a[C:] = ql
    lhs_b = np.zeros((KM, QROWS), np.float16)
    lhs_b[:C] = qh
    lhs_b[C : C + 3] = 1.0
    return {"lhs_a": lhs_a, "lhs_b": lhs_b, "rhs_a": rhs_a, "rhs_b": rhs_b}


def _merge_ranks(U, L):
    """Merge each row's UW device-selected candidates (values U, local idx L)
    into the top-18 global indices. Slot order within equal values == global
    index order, so a stable sort reproduces jax.lax.top_k tie-breaking.
    Returns (idx (R,18) int64, bad-row mask (R,))."""
    R = U.shape[0]
    g_of_slot = np.asarray(GROUP_BOUNDS[:-1], dtype=np.int64)[
        np.arange(UW) // 8
    ]
    Gidx = L.astype(np.int64) + g_of_slot[None, :]
    order = np.argsort(-U, axis=1, kind="stable")[:, :K_EFF]   # top-18 slots
    out = np.take_along_axis(Gidx, order, axis=1)
    # hazard: a group whose full top-8 landed in the top-18 may have lost a
    # 9th member that belongs there
    grp = order // 8
    counts = np.zeros((R, NG), np.int32)
    np.add.at(counts, (np.repeat(np.arange(R), K_EFF), grp.ravel()), 1)
    bad = (counts >= 8).any(axis=1)
    return out, bad


_NC_CACHE = {}


def kernel(x: np.ndarray) -> np.ndarray:
    x = np.asarray(x)
    assert x.shape == (B, C, N, 1), x.shape
    X = np.ascontiguousarray(np.transpose(x[..., 0], (0, 2, 1)))  # (B, N, C)

    if N_TILES not in _NC_CACHE:
        _NC_CACHE[N_TILES] = _build_program(N_TILES)
    nc = _NC_CACHE[N_TILES]

    in_maps = [_prep_core_inputs(X, c) for c in range(N_CORES)]
    res = run_bass_kernel_spmd(nc, in_maps, core_ids=list(range(N_CORES)))

    nn_idx = np.empty((B, N, K_EFF), np.int64)
    bad_rows = [[] for _ in range(B)]
    for core in range(N_CORES):
        b, h = divmod(core, N_CORES // B)
        r = res.results[core]
        idx, bad = _merge_ranks(r["u_out"], r["l_out"])
        nn_idx[b, h * QROWS : (h + 1) * QROWS] = idx
        if bad.any():
            bad_rows[b].extend((h * QROWS + np.nonzero(bad)[0]).tolist())

    # vectorized host repair of hazard rows (exact fp32 recompute)
    for b in range(B):
        if not bad_rows[b]:
            continue
        rows = np.asarray(sorted(bad_rows[b]))
        Xb = X[b]
        xsq = np.sum(Xb * Xb, axis=1, dtype=np.float32)
        S = (2.0 * Xb[rows]) @ Xb.T
        S = (S - xsq[None, :]).astype(np.float32)
        order = np.argsort(-S, axis=1, kind="stable")
        nn_idx[b, rows] = order[:, :K_EFF]

    nn_dil = nn_idx[:, :, ::DILATION]                       # (B, N, 9)
    center = np.broadcast_to(np.arange(N)[None, :, None], nn_dil.shape)
    out = np.stack((nn_dil, center), axis=0).astype(np.int32)
    return out



# revision 6
# speedup vs baseline: 1.6288x; 1.6288x over previous
"""TRN2 Bass kernel for DenseDilatedKnnGraph (B=4, C=64, N=4096, k=9, dilation=2).

Algorithm
---------
reference: xt (B,N,C); dist(i,j) = |xi|^2 - 2<xi,xj> + |xj|^2; nn_idx = top-18
of -dist per row (stable, lowest-index tie-break); output nn_idx[..., ::2] plus
a center-index row -> (2, B, N, 9) int32.

Per-row ordering of -dist is identical to the ordering of
    s_ij = 2<xi,xj> - |xj|^2
(the |xi|^2 term is constant per row).

Device (per core, SPMD over 8 cores; core = (batch, query-half)):
  - S computed via ONE fp16 K=67 matmul per 512-col chunk (stationary
    [qh(64); ones(3)], moving [ch(64); s1; s2; s3] where qh=fp16(2x_i),
    ch=fp16(x_j), s1..s3 = exact 3-level fp16 split of -|xj|^2).
    Values carry ~5e-3 absolute error from the fp16 rounding of q and c;
    selection errors this causes are detected host-side (margin flags)
    and repaired exactly.
  - DVE pass A: tensor_max pairs of PSUM columns (j, j+1024) per half
    -> SBUF fp32 [128,1024].  This is the cheapest possible crossing of
    the fp32 score stream (2 reads/lane/cycle).
  - GPSIMD pass B/C: continue the max tree 1024->512->256 per half ->
    one [128, 512] "slots" tile per query tile.  slot (h,j) = max of
    columns h*2048 + j + 256k, k=0..7.
  - DVE MAX8 (5 groups of ~102 slots) + MATCH/FIND_INDEX8 -> 40 slot ids
    per row (u16).  Only indices are DMA'd out.
  - Software-pipelined: the MAX8/FIND for tile t is emitted after pass A
    of tile t+1 so the DVE never stalls on the GPSIMD tree.

Host: each returned slot expands to its 8 member columns (320 candidates
per row); exact fp32 re-scoring + two-stage stable argsort reproduces the
jax top_k ordering (value desc, lowest index on ties).  Rows are repaired
by exact full recompute when (a) all 8 returned slots of some group score
>= v18 - eps (a 9th top-18 member may hide behind them), or (b) a FIND
duplicate collision at >= v18 - eps lost a slot.  Both checks follow from:
a true top-18 member can only be hidden by slots whose maxima are >= its
value (up to the device error bound eps).
"""

import numpy as np

import concourse.bacc as bacc
import concourse.mybir as mybir
import concourse.tile as tile
from concourse.bass_utils import run_bass_kernel_spmd

# Problem constants (hardcoded per harness contract).
B = 4
C = 64
N = 4096
K = 9
DILATION = 2
K_EFF = K * DILATION      # 18
P = 128                   # partitions / queries per tile
KM = C + 3                # matmul contraction: 64 q rows + 3 xsq rows
N_CORES = 8
QROWS = (B * N) // N_CORES          # 2048 query rows per core
N_TILES = QROWS // P                # 16 tiles per core

FMERGE = 8                # columns folded into one slot by the max tree
SLOTS = N // FMERGE       # 512 slots per row
HSLOT = SLOTS // 2        # 256 slots per half
# MAX8 group boundaries over slots.
GROUP_BOUNDS = (0, 103, 205, 308, 410, 512)
NG = len(GROUP_BOUNDS) - 1
UW = NG * 8               # selected slots per row (40)
EPS = 0.06                # device value error bound for host flags
C0 = 96.0                 # score bias: centers top scores near 0 for fp16


def _build_program(n_tiles=N_TILES):
    nc = bacc.Bacc(
        "TRN2", target_bir_lowering=False, debug=False, enable_asserts=False
    )
    f32 = mybir.dt.float32
    f16 = mybir.dt.float16
    u16 = mybir.dt.uint16
    nq = n_tiles * P
    lhs = nc.dram_tensor("lhs", (KM, nq), f16, kind="ExternalInput")
    rhs = nc.dram_tensor("rhs", (KM, N), f16, kind="ExternalInput")
    l_out = nc.dram_tensor("l_out", (nq, UW), u16, kind="ExternalOutput")
    lhs_ap, rhs_ap, l_ap = lhs.ap(), rhs.ap(), l_out.ap()

    with tile.TileContext(nc) as tc:
        with (
            tc.tile_pool(name="const", bufs=1) as cpool,
            tc.tile_pool(name="psum", bufs=2, space="PSUM") as ppool,
            tc.tile_pool(name="sbf", bufs=4) as sbpool,
            tc.tile_pool(name="m1p", bufs=4) as m1pool,
            tc.tile_pool(name="m2p", bufs=4) as m2pool,
            tc.tile_pool(name="slotp", bufs=3) as spool,
            tc.tile_pool(name="outp", bufs=3) as opool,
        ):
            # dependency-free warm-up matmuls that run during the input-DMA
            # prologue (nudges the PE toward its full-rate mode)
            prime = cpool.tile([KM, 512], f16)
            nc.gpsimd.memset(prime[:, :], 0.0)
            pps = ppool.tile([P, N // 2], f32, tag="ps")
            for _ in range(12):
                nc.tensor.matmul(pps[:, :512], prime[:, :128], prime[:, :],
                                 start=True, stop=True)

            # per-512-column-chunk rhs tiles: the first matmul only waits
            # for its own chunk, not the whole load
            r_sb = [
                cpool.tile([KM, 512], f16, name=f"r{j}", tag=f"r{j}")
                for j in range(8)
            ]
            l_sb = cpool.tile([KM, nq], f16)
            w0 = min(512, nq)
            nc.sync.dma_start(l_sb[:, 0:w0], lhs_ap[:, 0:w0])
            for j in range(8):
                nc.sync.dma_start(r_sb[j][:, :], rhs_ap[:, j * 512 : (j + 1) * 512])
            for j in range(512, nq, 512):
                w = min(512, nq - j)
                nc.sync.dma_start(l_sb[:, j : j + w], lhs_ap[:, j : j + w])

            def emit_maxfind(t, slot_t):
                u = opool.tile([P, UW], f16, tag="u")
                lo = opool.tile([P, UW], u16, tag="l")
                for g in range(NG):
                    nc.vector.max(
                        out=u[:, g * 8 : (g + 1) * 8],
                        in_=slot_t[:, GROUP_BOUNDS[g] : GROUP_BOUNDS[g + 1]],
                    )
                for g in range(NG):
                    nc.vector.max_index(
                        out=lo[:, g * 8 : (g + 1) * 8],
                        in_max=u[:, g * 8 : (g + 1) * 8],
                        in_values=slot_t[:, GROUP_BOUNDS[g] : GROUP_BOUNDS[g + 1]],
                    )
                rs = slice(t * P, (t + 1) * P)
                nc.sync.dma_start(l_ap[rs, :], lo[:])

            prev = None  # (t, slots tile) pending MAX8/FIND
            for t in range(n_tiles):
                qs = slice(t * P, (t + 1) * P)
                slot_t = spool.tile([P, SLOTS], f16, tag="slots")
                for h in range(2):
                    ps = ppool.tile([P, N // 2], f32, tag="ps")
                    for j in range(4):
                        cj = h * 4 + j
                        nc.tensor.matmul(
                            ps[:, j * 512 : (j + 1) * 512],
                            l_sb[:, qs], r_sb[cj][:, :],
                            start=True, stop=True,
                        )
                    # scalar engine stages the half into SBUF as fp16; the
                    # DVE max tree then runs in 2x (16-bit) mode
                    sbf = sbpool.tile([P, N // 2], f16, tag="sbf")
                    nc.scalar.copy(sbf[:, :], ps[:, :])
                    m1 = m1pool.tile([P, 1024], f16, tag="m1")
                    nc.vector.tensor_max(m1[:, :], sbf[:, 0:1024], sbf[:, 1024:2048])
                    m2 = m2pool.tile([P, 512], f16, tag="m2")
                    nc.vector.tensor_max(m2[:, :], m1[:, 0:512], m1[:, 512:1024])
                    nc.vector.tensor_max(
                        slot_t[:, h * HSLOT : (h + 1) * HSLOT],
                        m2[:, 0:256], m2[:, 256:512],
                    )
                if prev is not None:
                    emit_maxfind(*prev)
                prev = (t, slot_t)
            emit_maxfind(*prev)
    nc.compile()
    return nc


def _prep_core_inputs(X, core):
    """X: (B, N, C) fp32. Returns input map for one core."""
    b, h = divmod(core, N_CORES // B)
    Xb = X[b]
    xsq = np.sum(Xb * Xb, axis=1, dtype=np.float32)
    # 3-level fp16 split of (C0 - xsq) (exact to ~1e-6); the C0 bias
    # centers the top scores near 0 where the fp16 grid is finest
    t0 = C0 - xsq
    s1 = t0.astype(np.float16)
    r = t0 - s1.astype(np.float32)
    s2 = r.astype(np.float16)
    s3 = (r - s2.astype(np.float32)).astype(np.float16)
    rhs = np.empty((KM, N), np.float16)
    rhs[:C] = Xb.T.astype(np.float16)
    rhs[C] = s1
    rhs[C + 1] = s2
    rhs[C + 2] = s3
    lhs = np.empty((KM, QROWS), np.float16)
    lhs[:C] = (2.0 * Xb[h * QROWS : (h + 1) * QROWS]).T.astype(np.float16)
    lhs[C:] = 1.0
    return {"lhs": lhs, "rhs": rhs}


# base slot id of the group each of the UW output columns belongs to
_GROUP_BASE = np.asarray(GROUP_BOUNDS[:-1], dtype=np.int64)[np.arange(UW) // 8]


def _merge_core(L, Xb, xsq, h):
    """L: (QROWS, UW) u16 slot-local ids for one core. Returns
    (top18 (QROWS,18) int64, flagged row mask (QROWS,))."""
    R = L.shape[0]
    slots = L.astype(np.int64) + _GROUP_BASE[None, :]          # (R, UW)
    base_col = (slots >> 8) * 2048 + (slots & 255)             # (R, UW)
    cand = (base_col[:, :, None] + 256 * np.arange(FMERGE)[None, None, :]
            ).reshape(R, UW * FMERGE)                          # (R, 320)
    Q = 2.0 * Xb[h * QROWS : (h + 1) * QROWS]                  # (R, C)
    # exact scores: v[r,m] = <Q[r], X[cand]> - xsq[cand], chunked gather
    v = np.empty((R, UW * FMERGE), np.float32)
    CH = 512
    for r0 in range(0, R, CH):
        r1 = min(r0 + CH, R)
        Xg = Xb[cand[r0:r1]]                                   # (ch, 320, C)
        v[r0:r1] = np.matmul(Xg, Q[r0:r1, :, None])[..., 0]
    v -= xsq[cand]

    # stable jax-style ordering: by value desc, lowest column id on ties
    ord1 = np.argsort(cand, axis=1, kind="stable")
    cand1 = np.take_along_axis(cand, ord1, axis=1)
    v1 = np.take_along_axis(v, ord1, axis=1)
    ord2 = np.argsort(-v1, axis=1, kind="stable")
    top = np.take_along_axis(cand1, ord2, axis=1)[:, :K_EFF]
    v18 = np.take_along_axis(v1, ord2, axis=1)[:, K_EFF - 1]

    # flags
    slotmax = v.reshape(R, UW, FMERGE).max(axis=2)             # (R, UW)
    thr = (v18 - EPS)[:, None]
    cnt = (slotmax >= thr).reshape(R, NG, 8).sum(axis=2)       # (R, NG)
    flag_count = (cnt >= 8).any(axis=1)
    s_sorted = np.sort(slots.reshape(R, NG, 8), axis=2)
    sm_sorted = np.take_along_axis(
        slotmax.reshape(R, NG, 8), np.argsort(slots.reshape(R, NG, 8), axis=2), axis=2
    )
    dup = (np.diff(s_sorted, axis=2) == 0) & (sm_sorted[:, :, 1:] >= thr[:, :, None])
    flag_dup = dup.any(axis=(1, 2))
    return top, flag_count | flag_dup


_NC_CACHE = {}


def kernel(x: np.ndarray) -> np.ndarray:
    x = np.asarray(x)
    assert x.shape == (B, C, N, 1), x.shape
    X = np.ascontiguousarray(np.transpose(x[..., 0], (0, 2, 1)))  # (B, N, C)

    if N_TILES not in _NC_CACHE:
        _NC_CACHE[N_TILES] = _build_program(N_TILES)
    nc = _NC_CACHE[N_TILES]

    in_maps = [_prep_core_inputs(X, c) for c in range(N_CORES)]
    res = run_bass_kernel_spmd(nc, in_maps, core_ids=list(range(N_CORES)))

    xsqs = [np.sum(X[b] * X[b], axis=1, dtype=np.float32) for b in range(B)]
    nn_idx = np.empty((B, N, K_EFF), np.int64)
    bad_rows = [[] for _ in range(B)]
    for core in range(N_CORES):
        b, h = divmod(core, N_CORES // B)
        L = res.results[core]["l_out"]
        idx, bad = _merge_core(L, X[b], xsqs[b], h)
        nn_idx[b, h * QROWS : (h + 1) * QROWS] = idx
        if bad.any():
            bad_rows[b].extend((h * QROWS + np.nonzero(bad)[0]).tolist())

    # vectorized host repair of flagged rows (exact fp32 recompute)
    for b in range(B):
        if not bad_rows[b]:
            continue
        rows = np.asarray(sorted(bad_rows[b]))
        Xb = X[b]
        S = (2.0 * Xb[rows]) @ Xb.T
        S = (S - xsqs[b][None, :]).astype(np.float32)
        order = np.argsort(-S, axis=1, kind="stable")
        nn_idx[b, rows] = order[:, :K_EFF]

    nn_dil = nn_idx[:, :, ::DILATION]                       # (B, N, 9)
    center = np.broadcast_to(np.arange(N)[None, :, None], nn_dil.shape)
    out = np.stack((nn_dil, center), axis=0).astype(np.int32)
    return out
